# revision 3
# baseline (speedup 1.0000x reference)
"""Trainium2 Bass kernel for nn_MixingNetwork (QMIX-style mixer).

Math (per sample b):
  hid_w = (state @ W_hw).reshape(H, N); out_w = state @ W_ow; hid_b = state @ W_hb
  h     = elu(hid_w @ q + hid_b);      q_tot = out_w . h + state @ W_ob (+ biases)

Strategy: pure data parallel over batch (512 samples/core on 8 cores).
The dominant contraction state @ W_hw runs in bf16 on the PE. W_hw columns
are host-permuted to n-outer order (col = n*H + h) so the per-sample
q-weighting becomes a per-partition-scalar multiply on the Scalar engine
(activation Copy with scale=q[:, n]), and the sum over the 64 agents is a
strided tensor_reduce on the Vector engine, grouped to pipeline with the PE.
All biases are folded in as rank-1 / small matmuls accumulated in PSUM.
"""

import numpy as np
import ml_dtypes

B, N_AGENTS, HIDDEN, STATE_DIM = 4096, 64, 256, 512
N_CORES = 8
BS = B // N_CORES          # samples per core
NBT = BS // 128            # b-tiles per core
KT = STATE_DIM // 128      # k-tiles over state dim
FCHUNK = 512               # columns of W per PSUM chunk
NF = HIDDEN * N_AGENTS // FCHUNK   # 32 chunks
NPC = FCHUNK // HIDDEN     # agents (n) per chunk = 2
GROUP = 8                  # f-chunks per partial-reduce group
NG = NF // GROUP           # groups

_CACHE = {}


def _emit_body(nc, tc, ctx, tile, mybir, dram, pools):
    bass = pools["bass"]
    bf16 = mybir.dt.bfloat16
    f32 = mybir.dt.float32
    AX = mybir.AxisListType
    ALU = mybir.AluOpType
    ACTF = mybir.ActivationFunctionType

    stateT, q, qT, w_perm, w_small, bmat, bias_small, out = dram
    consts, wpool, spool, papool, hpool, pspool, smpool = (
        pools["consts"], pools["w"], pools["s"], pools["pa"], pools["h"],
        pools["ps"], pools["sm"],
    )

    # ---- constants into SBUF (emitted per rep; cheap) ----
    stateT_sb = consts.tile([128, KT, BS], bf16, tag="stateT")
    nc.gpsimd.dma_start(stateT_sb[:], stateT.rearrange("(k p) b -> p k b", p=128))
    q_sb = consts.tile([128, NBT, N_AGENTS], f32, tag="q")
    nc.gpsimd.dma_start(q_sb[:], q.rearrange("(t p) n -> p t n", p=128))
    qT_sb = consts.tile([N_AGENTS, BS], bf16, tag="qT")
    nc.gpsimd.dma_start(qT_sb[:], qT[:, :])
    wsm_sb = consts.tile([128, KT, HIDDEN * 2 + 1], bf16, tag="wsm")
    nc.gpsimd.dma_start(wsm_sb[:], w_small.rearrange("(k p) c -> p k c", p=128))
    bmat_sb = consts.tile([N_AGENTS, HIDDEN], bf16, tag="bmat")
    nc.gpsimd.dma_start(bmat_sb[:], bmat[:, :])
    bias_sb = consts.tile([1, HIDDEN * 2 + 1], bf16, tag="bias")
    nc.gpsimd.dma_start(bias_sb[:], bias_small[:, :])
    ones_sb = consts.tile([1, 128], bf16, tag="ones")
    nc.vector.memset(ones_sb[:], 1.0)

    partials = [papool.tile([128, NG, HIDDEN], f32, tag=f"pa{bt}", name=f"pa{bt}")
                for bt in range(NBT)]
    S = [None] * NBT

    # ---- big contraction: G = stateT.T @ W_perm, scaled by q, reduced over n ----
    for f in range(NF):
        wt = wpool.tile([128, KT, FCHUNK], bf16, tag="w")
        nc.gpsimd.dma_start(
            wt[:],
            w_perm.rearrange("(k p) n -> p k n", p=128)[:, :, f * FCHUNK:(f + 1) * FCHUNK],
        )
        g, pos = divmod(f, GROUP)
        for bt in range(NBT):
            if pos == 0:
                S[bt] = spool.tile([128, NPC * GROUP, HIDDEN], bf16, tag=f"S{bt}",
                                   name=f"S{bt}")
            ps = pspool.tile([128, FCHUNK], f32, tag="ps")
            bsl = slice(bt * 128, (bt + 1) * 128)
            for k in range(KT):
                nc.tensor.matmul(
                    ps[:], stateT_sb[:, k, bsl], wt[:, k, :],
                    start=(k == 0), stop=(k == KT - 1),
                )
            for j in range(NPC):
                n_local = NPC * pos + j
                n_glob = NPC * f + j
                nc.scalar.activation(
                    S[bt][:, n_local, :], ps[:, j * HIDDEN:(j + 1) * HIDDEN],
                    ACTF.Copy, scale=q_sb[:, bt, n_glob:n_glob + 1],
                )
            if pos == GROUP - 1:
                nc.vector.tensor_reduce(
                    partials[bt][:, g, :],
                    S[bt][:].rearrange("p n h -> p h n"),
                    axis=AX.X, op=ALU.add,
                )

    # ---- per-b-tile tail: hypernet biases + small matmuls + ELU + final dot ----
    for bt in range(NBT):
        bsl = slice(bt * 128, (bt + 1) * 128)
        hs = smpool.tile([128, HIDDEN], f32, tag="hsum")
        for k in range(KT):
            nc.tensor.matmul(hs[:], stateT_sb[:, k, bsl], wsm_sb[:, k, 0:HIDDEN],
                             start=(k == 0), stop=False)
        nc.tensor.matmul(hs[:], qT_sb[:, bsl], bmat_sb[:], start=False, stop=False)
        nc.tensor.matmul(hs[:], ones_sb[:, 0:128], bias_sb[:, 0:HIDDEN],
                         start=False, stop=True)

        ow = smpool.tile([128, HIDDEN], f32, tag="ow")
        for k in range(KT):
            nc.tensor.matmul(ow[:], stateT_sb[:, k, bsl],
                             wsm_sb[:, k, HIDDEN:2 * HIDDEN],
                             start=(k == 0), stop=False)
        nc.tensor.matmul(ow[:], ones_sb[:, 0:128], bias_sb[:, HIDDEN:2 * HIDDEN],
                         start=False, stop=True)

        ob = smpool.tile([128, 1], f32, tag="ob")
        for k in range(KT):
            nc.tensor.matmul(ob[:], stateT_sb[:, k, bsl],
                             wsm_sb[:, k, 2 * HIDDEN:2 * HIDDEN + 1],
                             start=(k == 0), stop=False)
        nc.tensor.matmul(ob[:], ones_sb[:, 0:128], bias_sb[:, 2 * HIDDEN:2 * HIDDEN + 1],
                         start=False, stop=True)

        hpre = hpool.tile([128, HIDDEN], f32, tag="hpre")
        nc.vector.tensor_reduce(
            hpre[:], partials[bt][:].rearrange("p g h -> p h g"), axis=AX.X, op=ALU.add,
        )
        hp = hpool.tile([128, HIDDEN], f32, tag="hp")
        nc.vector.tensor_add(hp[:], hpre[:], hs[:])
        # elu(x) = max(x,0) + exp(min(x,0)) - 1
        t0 = hpool.tile([128, HIDDEN], f32, tag="t0")
        nc.vector.tensor_scalar_min(t0[:], hp[:], 0.0)
        ex = hpool.tile([128, HIDDEN], f32, tag="ex")
        nc.scalar.activation(ex[:], t0[:], ACTF.Exp)
        t1 = hpool.tile([128, HIDDEN], f32, tag="t1")
        nc.vector.tensor_scalar_max(t1[:], hp[:], 0.0)
        h2 = hpool.tile([128, HIDDEN], f32, tag="h2")
        nc.vector.tensor_add(h2[:], t1[:], ex[:])
        h3 = hpool.tile([128, HIDDEN], f32, tag="h3")
        nc.vector.tensor_scalar_add(h3[:], h2[:], -1.0)

        scr = hpool.tile([128, HIDDEN], f32, tag="scr")
        nc.vector.tensor_mul(scr[:], h3[:], ow[:])
        qts = hpool.tile([128, 1], f32, tag="qts")
        nc.vector.tensor_reduce(qts[:], scr[:], axis=AX.X, op=ALU.add)
        qt = hpool.tile([128, 1], f32, tag="qt")
        nc.vector.tensor_add(qt[:], qts[:], ob[:, 0:1])
        nc.gpsimd.dma_start(out[bsl, :], qt[:])


def build_module(reps=1):
    """Build and compile the per-core Bass module. reps>1 repeats the whole
    computation in one NEFF (for timing)."""
    from contextlib import ExitStack
    import concourse.bass as bass
    import concourse.tile as tile
    from concourse import bacc, mybir

    bf16 = mybir.dt.bfloat16
    f32 = mybir.dt.float32

    nc = bacc.Bacc("TRN2", target_bir_lowering=False)
    stateT = nc.dram_tensor("stateT", [STATE_DIM, BS], bf16, kind="ExternalInput").ap()
    q = nc.dram_tensor("q", [BS, N_AGENTS], f32, kind="ExternalInput").ap()
    qT = nc.dram_tensor("qT", [N_AGENTS, BS], bf16, kind="ExternalInput").ap()
    w_perm = nc.dram_tensor("w_perm", [STATE_DIM, HIDDEN * N_AGENTS], bf16,
                            kind="ExternalInput").ap()
    w_small = nc.dram_tensor("w_small", [STATE_DIM, HIDDEN * 2 + 1], bf16,
                             kind="ExternalInput").ap()
    bmat = nc.dram_tensor("bmat", [N_AGENTS, HIDDEN], bf16, kind="ExternalInput").ap()
    bias_small = nc.dram_tensor("bias_small", [1, HIDDEN * 2 + 1], bf16,
                                kind="ExternalInput").ap()
    out = nc.dram_tensor("out", [BS, 1], f32, kind="ExternalOutput").ap()
    dram = (stateT, q, qT, w_perm, w_small, bmat, bias_small, out)

    with tile.TileContext(nc) as tc:
        with ExitStack() as ctx:
            pools = {
                "bass": bass,
                "consts": ctx.enter_context(tc.tile_pool(name="consts", bufs=1)),
                "w": ctx.enter_context(tc.tile_pool(name="w", bufs=3)),
                "s": ctx.enter_context(tc.tile_pool(name="s", bufs=2)),
                "pa": ctx.enter_context(tc.tile_pool(name="pa", bufs=1)),
                "h": ctx.enter_context(tc.tile_pool(name="h", bufs=2)),
                "ps": ctx.enter_context(tc.tile_pool(name="ps", bufs=4, space="PSUM")),
                "sm": ctx.enter_context(tc.tile_pool(name="sm", bufs=1, space="PSUM")),
            }
            for _ in range(reps):
                _emit_body(nc, tc, ctx, tile, mybir, dram, pools)
    nc.compile()
    return nc


def make_in_maps(q_values, state_representation, W_hw, b_hw, W_ow, b_ow, W_hb,
                 b_hb, W_ob, b_ob):
    bf16 = ml_dtypes.bfloat16
    q = np.asarray(q_values, dtype=np.float32).reshape(B, N_AGENTS)
    st = np.asarray(state_representation, dtype=np.float32)
    W_hw = np.asarray(W_hw, dtype=np.float32)
    # permute columns of W_hw from (h, n) to (n, h) order
    w_perm = np.ascontiguousarray(
        W_hw.reshape(STATE_DIM, HIDDEN, N_AGENTS).transpose(0, 2, 1)
        .reshape(STATE_DIM, HIDDEN * N_AGENTS)).astype(bf16)
    w_small = np.ascontiguousarray(np.concatenate(
        [np.asarray(W_hb, np.float32), np.asarray(W_ow, np.float32),
         np.asarray(W_ob, np.float32)], axis=1)).astype(bf16)
    bmat = np.ascontiguousarray(
        np.asarray(b_hw, np.float32).reshape(HIDDEN, N_AGENTS).T).astype(bf16)
    bias_small = np.concatenate(
        [np.asarray(b_hb, np.float32), np.asarray(b_ow, np.float32),
         np.asarray(b_ob, np.float32)]).reshape(1, HIDDEN * 2 + 1).astype(bf16)
    in_maps = []
    for c in range(N_CORES):
        sl = slice(c * BS, (c + 1) * BS)
        in_maps.append({
            "stateT": np.ascontiguousarray(st[sl].T).astype(bf16),
            "q": np.ascontiguousarray(q[sl]),
            "qT": np.ascontiguousarray(q[sl].T).astype(bf16),
            "w_perm": w_perm,
            "w_small": w_small,
            "bmat": bmat,
            "bias_small": bias_small,
        })
    return in_maps


def kernel(**inputs):
    from concourse.bass_utils import run_bass_kernel_spmd

    if "nc" not in _CACHE:
        _CACHE["nc"] = build_module()
    nc = _CACHE["nc"]
    in_maps = make_in_maps(**inputs)
    res = run_bass_kernel_spmd(nc, in_maps, core_ids=list(range(N_CORES)))
    out = np.concatenate([res.results[c]["out"] for c in range(N_CORES)], axis=0)
    return out.astype(np.float32)


# revision 6
# speedup vs baseline: 282.5561x; 282.5561x over previous
"""Trainium2 Bass kernel for nn_MixingNetwork (QMIX-style mixer).

Math (per sample b):
  hid_w = (state @ W_hw).reshape(H, N); out_w = state @ W_ow; hid_b = state @ W_hb
  h     = elu(hid_w @ q + hid_b);      q_tot = out_w . h + state @ W_ob (+ biases)

Strategy: pure data parallel over batch (512 samples/core on 8 cores).
The dominant contraction state @ W_hw runs in bf16 on the PE. W_hw columns
are host-permuted to n-outer order (col = n*H + h) so the per-sample
q-weighting becomes a per-partition-scalar multiply on the Scalar engine
(activation Copy with scale=q[:, n]), and the sum over the 64 agents is a
strided tensor_reduce on the Vector engine, grouped to pipeline with the PE.
All biases are folded in as rank-1 / small matmuls accumulated in PSUM.
"""

import numpy as np
import ml_dtypes

B, N_AGENTS, HIDDEN, STATE_DIM = 4096, 64, 256, 512
N_CORES = 8
BS = B // N_CORES          # samples per core
NBT = BS // 128            # b-tiles per core
KT = STATE_DIM // 128      # k-tiles over state dim
FCHUNK = 512               # columns of W per PSUM chunk
NF = HIDDEN * N_AGENTS // FCHUNK   # 32 chunks
NPC = FCHUNK // HIDDEN     # agents (n) per chunk = 2
GROUP = 8                  # f-chunks per partial-reduce group
NG = NF // GROUP           # groups

_CACHE = {}

# build-time tuning knobs (A/B testing)
CFG = {
    "dve_every": 4,      # every Nth scale op on DVE (0 = all on ACT)
    "sync_w_dma": True,  # W-chunk DMAs via HWDGE (sync engine)
    "ps_bufs": 4,        # big-psum pool bufs
    "w_bufs": 3,
}


def _emit_body(nc, tc, ctx, tile, mybir, dram, pools):
    bass = pools["bass"]
    bf16 = mybir.dt.bfloat16
    f32 = mybir.dt.float32
    AX = mybir.AxisListType
    ALU = mybir.AluOpType
    ACTF = mybir.ActivationFunctionType

    stateT, q, qT, w_perm, w_small, bmat, bias_small, out = dram
    consts, wpool, spool, papool, hpool, pspool, smpool = (
        pools["consts"], pools["w"], pools["s"], pools["pa"], pools["h"],
        pools["ps"], pools["sm"],
    )

    # ---- constants into SBUF (emitted per rep; cheap) ----
    stateT_sb = consts.tile([128, KT, BS], bf16, tag="stateT")
    nc.sync.dma_start(stateT_sb[:], stateT.rearrange("(k p) b -> p k b", p=128))
    q_sb = consts.tile([128, NBT, N_AGENTS], f32, tag="q")
    nc.gpsimd.dma_start(q_sb[:], q.rearrange("(t p) n -> p t n", p=128))
    qT_sb = consts.tile([N_AGENTS, BS], bf16, tag="qT")
    nc.gpsimd.dma_start(qT_sb[:], qT[:, :])
    wsm_sb = consts.tile([128, KT, HIDDEN * 2 + 1], bf16, tag="wsm")
    nc.gpsimd.dma_start(wsm_sb[:], w_small.rearrange("(k p) c -> p k c", p=128))
    bmat_sb = consts.tile([N_AGENTS, HIDDEN], bf16, tag="bmat")
    nc.gpsimd.dma_start(bmat_sb[:], bmat[:, :])
    bias_sb = consts.tile([1, HIDDEN * 2 + 1], bf16, tag="bias")
    nc.gpsimd.dma_start(bias_sb[:], bias_small[:, :])
    ones_sb = consts.tile([1, 128], bf16, tag="ones")
    nc.vector.memset(ones_sb[:], 1.0)

    partials = [papool.tile([128, NG, HIDDEN], f32, tag=f"pa{bt}", name=f"pa{bt}")
                for bt in range(NBT)]
    S = [None] * NBT

    # ---- big contraction: G = stateT.T @ W_perm, scaled by q, reduced over n ----
    # fraction of scale ops routed to DVE (rest on ACT) to balance engines
    ndve = 0
    for f in range(NF):
        wt = wpool.tile([128, KT, FCHUNK], bf16, tag="w")
        dma_eng = nc.sync if CFG["sync_w_dma"] else nc.gpsimd
        dma_eng.dma_start(
            wt[:],
            w_perm.rearrange("(k p) n -> p k n", p=128)[:, :, f * FCHUNK:(f + 1) * FCHUNK],
        )
        g, pos = divmod(f, GROUP)
        for bt in range(NBT):
            if pos == 0:
                S[bt] = spool.tile([128, NPC * GROUP, HIDDEN], bf16, tag=f"S{bt}",
                                   name=f"S{bt}")
            ps = pspool.tile([128, FCHUNK], f32, tag="ps")
            bsl = slice(bt * 128, (bt + 1) * 128)
            for k in range(KT):
                nc.tensor.matmul(
                    ps[:], stateT_sb[:, k, bsl], wt[:, k, :],
                    start=(k == 0), stop=(k == KT - 1),
                )
            for j in range(NPC):
                n_local = NPC * pos + j
                n_glob = NPC * f + j
                ndve += 1
                if CFG["dve_every"] and ndve % CFG["dve_every"] == 0:
                    nc.vector.tensor_scalar_mul(
                        S[bt][:, n_local, :], ps[:, j * HIDDEN:(j + 1) * HIDDEN],
                        q_sb[:, bt, n_glob:n_glob + 1],
                    )
                else:
                    nc.scalar.activation(
                        S[bt][:, n_local, :], ps[:, j * HIDDEN:(j + 1) * HIDDEN],
                        ACTF.Copy, scale=q_sb[:, bt, n_glob:n_glob + 1],
                    )
            if pos == GROUP - 1:
                nc.vector.tensor_reduce(
                    partials[bt][:, g, :],
                    S[bt][:].rearrange("p n h -> p h n"),
                    axis=AX.X, op=ALU.add,
                )

    # ---- per-b-tile tail: hypernet biases + small matmuls + ELU + final dot ----
    for bt in range(NBT):
        bsl = slice(bt * 128, (bt + 1) * 128)
        hs = smpool.tile([128, HIDDEN], f32, tag="hsum")
        for k in range(KT):
            nc.tensor.matmul(hs[:], stateT_sb[:, k, bsl], wsm_sb[:, k, 0:HIDDEN],
                             start=(k == 0), stop=False)
        nc.tensor.matmul(hs[:], qT_sb[:, bsl], bmat_sb[:], start=False, stop=False)
        nc.tensor.matmul(hs[:], ones_sb[:, 0:128], bias_sb[:, 0:HIDDEN],
                         start=False, stop=True)

        ow = smpool.tile([128, HIDDEN], f32, tag="ow")
        for k in range(KT):
            nc.tensor.matmul(ow[:], stateT_sb[:, k, bsl],
                             wsm_sb[:, k, HIDDEN:2 * HIDDEN],
                             start=(k == 0), stop=False)
        nc.tensor.matmul(ow[:], ones_sb[:, 0:128], bias_sb[:, HIDDEN:2 * HIDDEN],
                         start=False, stop=True)

        ob = smpool.tile([128, 1], f32, tag="ob")
        for k in range(KT):
            nc.tensor.matmul(ob[:], stateT_sb[:, k, bsl],
                             wsm_sb[:, k, 2 * HIDDEN:2 * HIDDEN + 1],
                             start=(k == 0), stop=False)
        nc.tensor.matmul(ob[:], ones_sb[:, 0:128], bias_sb[:, 2 * HIDDEN:2 * HIDDEN + 1],
                         start=False, stop=True)

        hpre = hpool.tile([128, HIDDEN], f32, tag="hpre")
        nc.vector.tensor_reduce(
            hpre[:], partials[bt][:].rearrange("p g h -> p h g"), axis=AX.X, op=ALU.add,
        )
        hp = hpool.tile([128, HIDDEN], f32, tag="hp")
        nc.vector.tensor_add(hp[:], hpre[:], hs[:])
        # elu(x) = max(x,0) + exp(min(x,0)) - 1
        t0 = hpool.tile([128, HIDDEN], f32, tag="t0")
        nc.vector.tensor_scalar_min(t0[:], hp[:], 0.0)
        ex = hpool.tile([128, HIDDEN], f32, tag="ex")
        nc.scalar.activation(ex[:], t0[:], ACTF.Exp)
        t1 = hpool.tile([128, HIDDEN], f32, tag="t1")
        nc.vector.tensor_scalar_max(t1[:], hp[:], 0.0)
        h2 = hpool.tile([128, HIDDEN], f32, tag="h2")
        nc.vector.tensor_add(h2[:], t1[:], ex[:])
        h3 = hpool.tile([128, HIDDEN], f32, tag="h3")
        nc.vector.tensor_scalar_add(h3[:], h2[:], -1.0)

        scr = hpool.tile([128, HIDDEN], f32, tag="scr")
        nc.vector.tensor_mul(scr[:], h3[:], ow[:])
        qts = hpool.tile([128, 1], f32, tag="qts")
        nc.vector.tensor_reduce(qts[:], scr[:], axis=AX.X, op=ALU.add)
        qt = hpool.tile([128, 1], f32, tag="qt")
        nc.vector.tensor_add(qt[:], qts[:], ob[:, 0:1])
        nc.gpsimd.dma_start(out[bsl, :], qt[:])


def build_module(reps=1):
    """Build and compile the per-core Bass module. reps>1 repeats the whole
    computation in one NEFF (for timing)."""
    from contextlib import ExitStack
    import concourse.bass as bass
    import concourse.tile as tile
    from concourse import bacc, mybir

    bf16 = mybir.dt.bfloat16
    f32 = mybir.dt.float32

    nc = bacc.Bacc("TRN2", target_bir_lowering=False)
    stateT = nc.dram_tensor("stateT", [STATE_DIM, BS], bf16, kind="ExternalInput").ap()
    q = nc.dram_tensor("q", [BS, N_AGENTS], f32, kind="ExternalInput").ap()
    qT = nc.dram_tensor("qT", [N_AGENTS, BS], bf16, kind="ExternalInput").ap()
    w_perm = nc.dram_tensor("w_perm", [STATE_DIM, HIDDEN * N_AGENTS], bf16,
                            kind="ExternalInput").ap()
    w_small = nc.dram_tensor("w_small", [STATE_DIM, HIDDEN * 2 + 1], bf16,
                             kind="ExternalInput").ap()
    bmat = nc.dram_tensor("bmat", [N_AGENTS, HIDDEN], bf16, kind="ExternalInput").ap()
    bias_small = nc.dram_tensor("bias_small", [1, HIDDEN * 2 + 1], bf16,
                                kind="ExternalInput").ap()
    out = nc.dram_tensor("out", [BS, 1], f32, kind="ExternalOutput").ap()
    dram = (stateT, q, qT, w_perm, w_small, bmat, bias_small, out)

    with tile.TileContext(nc) as tc:
        with ExitStack() as ctx:
            pools = {
                "bass": bass,
                "consts": ctx.enter_context(tc.tile_pool(name="consts", bufs=1)),
                "w": ctx.enter_context(tc.tile_pool(name="w", bufs=CFG["w_bufs"])),
                "s": ctx.enter_context(tc.tile_pool(name="s", bufs=2)),
                "pa": ctx.enter_context(tc.tile_pool(name="pa", bufs=1)),
                "h": ctx.enter_context(tc.tile_pool(name="h", bufs=2)),
                "ps": ctx.enter_context(tc.tile_pool(name="ps", bufs=CFG["ps_bufs"], space="PSUM")),
                "sm": ctx.enter_context(tc.tile_pool(name="sm", bufs=1, space="PSUM")),
            }
            for _ in range(reps):
                _emit_body(nc, tc, ctx, tile, mybir, dram, pools)
    nc.compile()
    return nc


def make_in_maps(q_values, state_representation, W_hw, b_hw, W_ow, b_ow, W_hb,
                 b_hb, W_ob, b_ob):
    bf16 = ml_dtypes.bfloat16
    q = np.asarray(q_values, dtype=np.float32).reshape(B, N_AGENTS)
    st = np.asarray(state_representation, dtype=np.float32)
    W_hw = np.asarray(W_hw, dtype=np.float32)
    # permute columns of W_hw from (h, n) to (n, h) order
    w_perm = np.ascontiguousarray(
        W_hw.reshape(STATE_DIM, HIDDEN, N_AGENTS).transpose(0, 2, 1)
        .reshape(STATE_DIM, HIDDEN * N_AGENTS)).astype(bf16)
    w_small = np.ascontiguousarray(np.concatenate(
        [np.asarray(W_hb, np.float32), np.asarray(W_ow, np.float32),
         np.asarray(W_ob, np.float32)], axis=1)).astype(bf16)
    bmat = np.ascontiguousarray(
        np.asarray(b_hw, np.float32).reshape(HIDDEN, N_AGENTS).T).astype(bf16)
    bias_small = np.concatenate(
        [np.asarray(b_hb, np.float32), np.asarray(b_ow, np.float32),
         np.asarray(b_ob, np.float32)]).reshape(1, HIDDEN * 2 + 1).astype(bf16)
    in_maps = []
    for c in range(N_CORES):
        sl = slice(c * BS, (c + 1) * BS)
        in_maps.append({
            "stateT": np.ascontiguousarray(st[sl].T).astype(bf16),
            "q": np.ascontiguousarray(q[sl]),
            "qT": np.ascontiguousarray(q[sl].T).astype(bf16),
            "w_perm": w_perm,
            "w_small": w_small,
            "bmat": bmat,
            "bias_small": bias_small,
        })
    return in_maps


def kernel(**inputs):
    from concourse.bass_utils import run_bass_kernel_spmd

    if "nc" not in _CACHE:
        _CACHE["nc"] = build_module()
    nc = _CACHE["nc"]
    in_maps = make_in_maps(**inputs)
    res = run_bass_kernel_spmd(nc, in_maps, core_ids=list(range(N_CORES)))
    out = np.concatenate([res.results[c]["out"] for c in range(N_CORES)], axis=0)
    return out.astype(np.float32)
